# revision 1
# baseline (speedup 1.0000x reference)
"""Fused multi-head cross-attention (single query per batch) + residual + LayerNorm
for Trainium2, data-parallel over batch across 8 NeuronCores.

Math (per batch row b):
    q = Wq @ queries[b] + bq                  (per head)
    k[n] = Wk @ keys[b,n]        (bias bk folded out: softmax shift-invariant)
    v[n] = Wv @ values[b,n]      (bias bv folded out: sum(a)=1 -> added to residual)
    a = softmax(q . k / sqrt(dph))
    out[b] = LayerNorm(concat_h(a . v) + queries[b] + bv_flat) * gamma + beta

Implementation notes:
  - All matmuls run in float32r (fp32 with 11-bit mantissa, 1 cycle/row on the
    PE at free-dim >= 256 -- 4x faster than fp32). Inputs pre-rounded on host.
  - Per core: 8 batch rows x 16 heads = 128 (b,h) pairs.
  - Every DRAM tensor is pre-tiled on the host into the exact SBUF layout so
    each loads with O(1) large DMAs (dma_start issue overhead is ~0.65us).
  - Projections produce k^T [he, n] and v [n, he] layouts directly so the
    attention matmuls contract on the partition dim with no device transposes
    of the big tensors (keys/values are transposed on the host).
  - Scores for one batch row: accumulate 8 matmuls with a block-diagonal
    masked-q stationary operand -> scores [16 heads, 512] in one PSUM tile.
  - attn.v: transposed softmax weights (PE transpose) as stationary, v as
    moving -> [8, 512] with valid head-blocks on the diagonal; a masked
    selector matmul then accumulates each row's diagonal into partition row b
    of the final [8, 1024] attention output (engines cannot address single
    partitions off 32-alignment, so the gather is done on the PE).
"""
import contextlib
import numpy as np
import concourse.bacc as bacc
import concourse.tile as tile
import concourse.mybir as mybir
import concourse.bass as bass
from concourse import bass_utils

B, N, D, H = 64, 512, 1024, 16
DPH = D // H            # 64
NCORES = 8
BL = B // NCORES        # 8 batch rows per core
NKV = BL * N            # 4096 key/value rows per core
DDT = D // 128          # 8 d-tiles
HET = D // 128          # 8 he-tiles
EPS = 1e-5
SCALE = 1.0 / np.sqrt(DPH)  # 0.125, exactly representable

f32 = mybir.dt.float32
f32r = mybir.dt.float32r
AF = mybir.ActivationFunctionType
AX = mybir.AxisListType


def _round_fp32r(x: np.ndarray) -> np.ndarray:
    """RNE round fp32 to 11 mantissa bits (matches walrus fp32_to_fp32r)."""
    x = np.ascontiguousarray(x, np.float32)
    b = x.view(np.uint32)
    bias = ((b >> 12) & np.uint32(1)) + np.uint32(0x7FF)
    return ((b + bias) & np.uint32(0xFFFFF000)).view(np.float32)


def _emit(nc, tc, ap, ctx, repeat=1):
    """Per-core program. `ap` maps dram tensor name -> AP."""
    const = ctx.enter_context(tc.tile_pool(name="const", bufs=1))
    wpool = ctx.enter_context(tc.tile_pool(name="weights", bufs=1))
    wqpool = ctx.enter_context(tc.tile_pool(name="wq", bufs=3))
    io = ctx.enter_context(tc.tile_pool(name="io", bufs=2))
    proj = ctx.enter_context(tc.tile_pool(name="proj", bufs=1))
    once = ctx.enter_context(tc.tile_pool(name="once", bufs=1))
    work = ctx.enter_context(tc.tile_pool(name="work", bufs=2))
    pp = ctx.enter_context(tc.tile_pool(name="pp", bufs=3, space="PSUM"))
    pscore = ctx.enter_context(tc.tile_pool(name="pscore", bufs=1, space="PSUM"))
    pat = ctx.enter_context(tc.tile_pool(name="pat", bufs=1, space="PSUM"))
    puv = ctx.enter_context(tc.tile_pool(name="puv", bufs=1, space="PSUM"))
    pfin = ctx.enter_context(tc.tile_pool(name="pfin", bufs=1, space="PSUM"))

    # ---- merged constants: one f32r blob + one f32 blob + epilogue blob ----
    # constR [128, 768]: qT [.,0:64] | mask [.,64:192] | dmask(p0-7) [.,192:704]
    #                    | sel(p0-7) [.,704:768]
    # constF [128, 24]:  bqT [.,0:8] | ident(p0-15) [.,8:24]
    cR = const.tile([128, 768], f32r, tag="cR")
    cF = const.tile([128, 24], f32, tag="cF")
    ep_sb = const.tile([BL, 3 * D], f32, tag="ep")            # qres|gamma|beta
    nc.sync.dma_start(cR[:], ap["constR"][:])
    nc.sync.dma_start(cF[:], ap["constF"][:])
    qT_sb = cR[:, 0:64]
    mask_sb = cR[:, 64:192]
    dmask_sb = cR[0:8, 192:704]
    sel_sb = cR[0:8, 704:768]
    bqT_sb = cF[:, 0:8]
    id_sb = cF[0:16, 8:24]
    qres_sb = ep_sb[:, 0:D]
    gam_sb = ep_sb[:, D:2 * D]
    bet_sb = ep_sb[:, 2 * D:3 * D]

    # preload the Sqrt activation table so the LN epilogue doesn't pay the
    # ~1.3us LoadActFuncSet on the critical tail
    warm = once.tile([1, 1], f32, tag="warm")
    nc.vector.memset(warm[:], 1.0)
    nc.scalar.activation(warm[:], warm[:], AF.Sqrt)

    # dummy matmuls on the const blob while the first kT/wk DMAs stream in:
    # ramps the PE HAM clock gate to 2.4 GHz before real work arrives
    pwarm = pp.tile([128, 512], f32, tag="pp")
    for i in range(24):
        nc.tensor.matmul(pwarm[:], cR[:, 0:128], cR[:, 256:768],
                         start=(i == 0), stop=(i == 23))
    wsink = once.tile([1, 1], f32, tag="wsink")
    nc.vector.tensor_copy(wsink[:], pwarm[0:1, 0:1])

    # ---- q projection + masked QB build (emitted inside b=0's iteration so
    # k-proj(b=0) leads the PE stream while wq chunks stream in) ----
    qb_sb = once.tile([128, HET * BL * H], f32r, tag="qb")

    def emit_qproj():
        q_sb = once.tile([BL, D], f32, tag="q_sb")
        ps0 = pp.tile([128, 512], f32, tag="pp")
        ps1 = pp.tile([128, 512], f32, tag="pp")
        for dd in range(DDT):
            wqt = wqpool.tile([128, D], f32r, tag="wqt")
            nc.sync.dma_start(wqt[:], ap["wqT"][dd * 128:(dd + 1) * 128, :])
            nc.tensor.matmul(ps0[0:BL, :], qT_sb[:, dd * BL:(dd + 1) * BL],
                             wqt[:, 0:512], start=(dd == 0), stop=(dd == DDT - 1))
            nc.tensor.matmul(ps1[0:BL, :], qT_sb[:, dd * BL:(dd + 1) * BL],
                             wqt[:, 512:1024], start=(dd == 0),
                             stop=(dd == DDT - 1))
        nc.vector.tensor_copy(q_sb[:, 0:512], ps0[0:BL, :])
        nc.vector.tensor_copy(q_sb[:, 512:1024], ps1[0:BL, :])

        # transpose q to [he, b] tiles; add bias; expand into masked QB.
        # QB layout: [128, j*128 + b*16 + h] (f32r), nonzero only when
        # h in {2j, 2j+1} matching partition half; values = SCALE * (q + bq).
        tmpq = once.tile([128, HET * BL], f32, tag="tmpq")
        for j in range(HET):
            pt = pp.tile([128, 512], f32, tag="pp")
            nc.tensor.transpose(pt[:, 0:BL], q_sb[:, j * 128:(j + 1) * 128],
                                id_sb[0:BL, 0:BL])
            nc.vector.tensor_scalar_add(tmpq[:, j * BL:(j + 1) * BL],
                                        pt[:, 0:BL], bqT_sb[:, j:j + 1])
        for j in range(HET):
            for b in range(BL):
                nc.vector.tensor_scalar_mul(
                    qb_sb[:, j * 128 + b * H:j * 128 + b * H + H],
                    mask_sb[:, j * H:(j + 1) * H],
                    tmpq[:, j * BL + b:j * BL + b + 1])

    # ---- resident weight buffers ----
    # wk layout [p, j*1024 + dd*128 + q] : k-proj group j needs only chunk j
    # wv layout [p, c*4096 + dd*512 + e] : v-proj (t,c) needs only chunk c
    wk_sb = wpool.tile([128, DDT * D], f32r, tag="wk")
    wv_sb = wpool.tile([128, DDT * D], f32r, tag="wv")

    # ---- b=0 inputs first, then weights ----
    kv_tiles = []

    def kv_dma(b):
        kT_in = io.tile([128, DDT * N], f32r, tag="kT_in")   # [p, dd*512+n]
        vT_in = io.tile([128, DDT * N], f32r, tag="vT_in")   # [p, dd*512+t*128+i]
        nc.sync.dma_start(kT_in[:], ap["kT"][:, b * 4096:(b + 1) * 4096])
        nc.sync.dma_start(vT_in[:], ap["vT"][:, b * 4096:(b + 1) * 4096])
        kv_tiles.append((kT_in, vT_in))

    def kv_dma_split(b):
        kT_in = io.tile([128, DDT * N], f32r, tag="kT_in")
        vT_in = io.tile([128, DDT * N], f32r, tag="vT_in")
        kv_tiles.append((kT_in, vT_in))
        return kT_in, vT_in

    kT0, vT0 = kv_dma_split(0)
    nc.sync.dma_start(kT0[:], ap["kT"][:, 0:4096])
    for j in range(4):
        nc.sync.dma_start(wk_sb[:, j * D:(j + 1) * D],
                          ap["wkT"][:, j * D:(j + 1) * D])
    nc.sync.dma_start(vT0[:], ap["vT"][:, 0:4096])
    for j in range(4, HET):
        nc.sync.dma_start(wk_sb[:, j * D:(j + 1) * D],
                          ap["wkT"][:, j * D:(j + 1) * D])

    # ---- final attention accumulators (held across the whole b loop) ----
    fin0 = pfin.tile([8, 512], f32, tag="fin0")
    fin1 = pfin.tile([8, 512], f32, tag="fin1")
    fins = [fin0, fin1]

    NITER = repeat * BL
    for it in range(NITER):
      b = it % BL
      if True:
        kT_in, vT_in = kv_tiles[it]

        # k projection -> k_projT [he (8 tiles), n=512]  (no bias: shift-inv.)
        k_projT = proj.tile([128, HET * N], f32r, tag="projbuf")
        for j in range(HET):
            ps = pp.tile([128, 512], f32, tag="pp")
            for dd in range(DDT):
                nc.tensor.matmul(
                    ps[:], wk_sb[:, j * D + dd * 128:j * D + (dd + 1) * 128],
                    kT_in[:, dd * N:(dd + 1) * N],
                    start=(dd == 0), stop=(dd == DDT - 1))
            nc.vector.tensor_copy(k_projT[:, j * N:(j + 1) * N], ps[:])

        if it == 0:
            emit_qproj()           # wq chunks stream while k-proj runs
            for c in range(2):     # wv needed from v-proj(b=0) onward
                nc.sync.dma_start(wv_sb[:, c * 4096:(c + 1) * 4096],
                                  ap["wvT"][:, c * 4096:(c + 1) * 4096])
            nc.sync.dma_start(ep_sb[:], ap["epi"][:])
        if it + 1 < NITER:
            kv_dma((it + 1) % BL)          # prefetch next batch row

        # scores [16, 512] = sum_j QB_j(b)^T @ k_projT_j   (scale folded in QB)
        sc = pscore.tile([16, N], f32, tag="sc")
        for j in range(HET):
            nc.tensor.matmul(
                sc[:], qb_sb[:, j * 128 + b * H:j * 128 + b * H + H],
                k_projT[:, j * N:(j + 1) * N],
                start=(j == 0), stop=(j == HET - 1))

        # softmax over n (free dim); weights normalized in place
        nmax = work.tile([16, 1], f32, tag="nmax")
        nc.vector.reduce_max(nmax[:], sc[:], axis=AX.X, negate=True)
        a_sb = work.tile([16, N], f32, tag="a_sb")
        ssum = work.tile([16, 1], f32, tag="ssum")
        nc.scalar.activation(a_sb[:], sc[:], AF.Exp,
                             bias=nmax[:], scale=1.0, accum_out=ssum[:])
        rsum = work.tile([16, 1], f32, tag="rsum")
        nc.vector.reciprocal(rsum[:], ssum[:])
        nc.vector.tensor_scalar_mul(a_sb[:], a_sb[:], rsum[:])

        # v projection -> v_proj [n (4 tiles of 128), he=1024]  (no bias)
        # he-chunk c innermost: each stationary vT tile is loaded once per dd
        # and reused for both he halves.
        v_proj = proj.tile([128, 4 * D], f32r, tag="projbuf")
        for t in range(4):
            psv0 = pp.tile([128, 512], f32, tag="pp")
            psv1 = pp.tile([128, 512], f32, tag="pp")
            psv = [psv0, psv1]
            for dd in range(DDT):
                for c in range(2):
                    nc.tensor.matmul(
                        psv[c][:],
                        vT_in[:, dd * N + t * 128:dd * N + (t + 1) * 128],
                        wv_sb[:, c * 4096 + dd * 512:c * 4096 + (dd + 1) * 512],
                        start=(dd == 0), stop=(dd == DDT - 1))
            for c in range(2):
                nc.vector.tensor_copy(
                    v_proj[:, t * D + c * 512:t * D + (c + 1) * 512], psv[c][:])

        # a^T via PE transpose: [n (4x128), 16 pairs]
        pt = pat.tile([128, 64], f32, tag="pt")
        for t in range(4):
            nc.tensor.transpose(pt[:, t * 16:(t + 1) * 16],
                                a_sb[:, t * 128:(t + 1) * 128], id_sb)
        aT_sb = work.tile([128, 64], f32r, tag="aT_sb")
        nc.vector.tensor_copy(aT_sb[:], pt[:])

        # attn.v: two head-groups of 8; diag-masked selector accumulates into
        # partition row b of the final [8, 1024] attention (cols = he).
        for g in range(2):
            pv = puv.tile([8, N], f32, tag="pv")
            for t in range(4):
                nc.tensor.matmul(
                    pv[:], aT_sb[:, t * 16 + g * 8:t * 16 + g * 8 + 8],
                    v_proj[:, t * D + g * 512:t * D + (g + 1) * 512],
                    start=(t == 0), stop=(t == 3))
            msked = work.tile([8, 512], f32r, tag="msked")
            nc.vector.tensor_mul(msked[:], pv[:], dmask_sb)
            nc.tensor.matmul(fins[g][:], sel_sb[:, b * 8:(b + 1) * 8],
                             msked[:], start=(it == 0),
                           stop=(it == NITER - 1))

    # ---- epilogue: residual + LayerNorm (in-place on x) ----
    x = once.tile([BL, D], f32, tag="x")
    nc.vector.tensor_add(x[:, 0:512], fin0[:], qres_sb[:, 0:512])
    nc.vector.tensor_add(x[:, 512:1024], fin1[:], qres_sb[:, 512:1024])

    msum = once.tile([BL, 1], f32, tag="msum")
    nc.vector.reduce_sum(msum[:], x[:], axis=AX.X)
    mu = once.tile([BL, 1], f32, tag="mu")
    nc.vector.tensor_scalar_mul(mu[:], msum[:], 1.0 / D)
    nc.vector.tensor_scalar_sub(x[:], x[:], mu[:])
    sq = once.tile([BL, D], f32, tag="q_sb")  # reuse dead q_sb slot
    vsum = once.tile([BL, 1], f32, tag="vsum")
    nc.scalar.activation(sq[:], x[:], AF.Square, accum_out=vsum[:])
    epst = once.tile([BL, 1], f32, tag="epst")
    nc.vector.memset(epst[:], EPS)
    std = once.tile([BL, 1], f32, tag="std")
    nc.scalar.activation(std[:], vsum[:], AF.Sqrt, scale=1.0 / D, bias=epst[:])
    rstd = once.tile([BL, 1], f32, tag="rstd")
    nc.vector.reciprocal(rstd[:], std[:])
    nc.vector.tensor_scalar_mul(x[:], x[:], rstd[:])
    nc.vector.tensor_mul(x[:], x[:], gam_sb)
    nc.vector.tensor_add(x[:], x[:], bet_sb)
    nc.sync.dma_start(ap["out"][:], x[:])


_CACHED = {}


def _build(repeat=1):
    global _CACHED
    if repeat in _CACHED:
        return _CACHED[repeat]
    nc = bacc.Bacc("TRN2", target_bir_lowering=False, debug=False,
                   num_devices=NCORES)
    names = {}
    def di(name, shape, dt):
        names[name] = nc.dram_tensor(name, shape, dt, kind="ExternalInput").ap()
    di("kT", [128, BL * DDT * N], f32r)
    di("vT", [128, BL * DDT * N], f32r)
    di("wkT", [128, DDT * D], f32r)
    di("wvT", [128, DDT * D], f32r)
    di("wqT", [D, D], f32r)
    di("constR", [128, 768], f32r)
    di("constF", [128, 24], f32)
    di("epi", [BL, 3 * D], f32)
    names["out"] = nc.dram_tensor("out", [BL, D], f32, kind="ExternalOutput").ap()
    with tile.TileContext(nc) as tc:
        with contextlib.ExitStack() as ctx:
            _emit(nc, tc, names, ctx, repeat=repeat)
    nc.compile()
    _CACHED[repeat] = nc
    return nc


def _host_prep(queries, keys, values, Wq, bq, Wk, bk, Wv, bv, gamma, beta):
    """Returns the per-core in_maps list (all tensors pre-tiled to SBUF layout)."""
    queries = np.asarray(queries, np.float32)
    keys = np.asarray(keys, np.float32)
    values = np.asarray(values, np.float32)
    wq_f = np.asarray(Wq, np.float32).reshape(D, D)   # [he, d]
    wk_f = np.asarray(Wk, np.float32).reshape(D, D)
    wv_f = np.asarray(Wv, np.float32).reshape(D, D)
    bq_f = np.asarray(bq, np.float32).reshape(D)
    bv_f = np.asarray(bv, np.float32).reshape(D)
    gamma = np.asarray(gamma, np.float32).reshape(D)
    beta = np.asarray(beta, np.float32).reshape(D)

    # wq: [d, he] (row chunks per d-tile are contiguous)
    wqT = _round_fp32r(wq_f.T)
    # wk: [p, j*1024 + dd*128 + q] = wk_f[j*128+q, dd*128+p]
    wkT = _round_fp32r(
        wk_f.reshape(HET, 128, DDT, 128).transpose(3, 0, 2, 1).reshape(128, -1))
    # wv: [p, c*4096 + dd*512 + e] = wv_f[c*512+e, dd*128+p]
    wvT = _round_fp32r(
        wv_f.reshape(2, 512, DDT, 128).transpose(3, 0, 2, 1).reshape(128, -1))

    mask = np.zeros((D, H), np.float32)
    for h in range(H):
        mask[h * DPH:(h + 1) * DPH, h] = SCALE
    mask_t = mask.reshape(DDT, 128, H).transpose(1, 0, 2).reshape(128, -1)
    bqT_t = bq_f.reshape(DDT, 128).T                         # [128, 8]
    dmask = np.zeros((8, 512), np.float32)
    for j in range(8):
        dmask[j, j * 64:(j + 1) * 64] = 1.0
    sel = np.zeros((8, 64), np.float32)
    for b in range(8):
        sel[:, b * 8 + b] = 1.0
    ident = np.eye(16, dtype=np.float32)

    constF = np.zeros((128, 24), np.float32)
    constF[:, 0:8] = bqT_t
    constF[0:16, 8:24] = ident

    in_maps = []
    for c in range(NCORES):
        sl = slice(c * BL, (c + 1) * BL)
        # kT: [p, b*4096 + dd*512 + n] = keys[b, n, dd*128+p]
        kT = _round_fp32r(
            keys[sl].reshape(BL, N, DDT, 128).transpose(3, 0, 2, 1)
            .reshape(128, -1))
        # vT: [p, b*4096 + dd*512 + t*128 + i] = values[b, t*128+i, dd*128+p]
        vT = _round_fp32r(
            values[sl].reshape(BL, 4, 128, DDT, 128).transpose(4, 0, 3, 1, 2)
            .reshape(128, -1))
        # qT: [p, dd*8 + b] = queries[b, dd*128+p]
        qT = _round_fp32r(
            queries[sl].reshape(BL, DDT, 128).transpose(2, 1, 0).reshape(128, -1))
        constR = np.zeros((128, 768), np.float32)
        constR[:, 0:64] = qT
        constR[:, 64:192] = mask_t
        constR[0:8, 192:704] = dmask
        constR[0:8, 704:768] = sel
        epi = np.concatenate(
            [queries[sl] + bv_f[None, :],
             np.tile(gamma[None, :], (BL, 1)),
             np.tile(beta[None, :], (BL, 1))], axis=1)
        in_maps.append({
            "wkT": wkT, "wvT": wvT, "wqT": wqT,
            "constR": constR, "constF": constF,
            "kT": kT, "vT": vT,
            "epi": np.ascontiguousarray(epi),
        })
    return in_maps


def kernel(queries, keys, values, Wq, bq, Wk, bk, Wv, bv, gamma, beta):
    nc = _build()
    in_maps = _host_prep(queries, keys, values, Wq, bq, Wk, bk, Wv, bv,
                         gamma, beta)
    last_err = None
    for attempt in range(3):
        try:
            res = bass_utils.run_bass_kernel_spmd(nc, in_maps,
                                                  core_ids=list(range(NCORES)))
            return np.concatenate([r["out"] for r in res.results], axis=0)
        except Exception as e:  # transient NRT device errors: retry
            last_err = e
            import time as _time
            _time.sleep(5)
    raise last_err



# revision 3
# speedup vs baseline: 6.2919x; 6.2919x over previous
"""Fused multi-head cross-attention (single query per batch) + residual + LayerNorm
for Trainium2, data-parallel over batch across 8 NeuronCores.

Algebraic restructuring vs the direct form: keys/values are NEVER projected.
    scores[b,h,n] = (Wq_h q_b + bq_h) . (Wk_h k_{b,n})  (bk drops: shift-inv)
                  = u[b,h] . k_{b,n},   u[b,h] = Wk_h^T (Wq_h q_b + bq_h)
    attn[b,h]     = Wv_h (sum_n a_n k?v_{b,n})          (bv folded into residual)
so the only O(N*D) work per row is two rank-contractions against the raw
keys/values streams (20x fewer MACs than projecting K/V).  Kernel becomes
HBM-bound: keys/values stream in fp8 (e4m3) with DoubleRow matmuls; all
accumulation in fp32 PSUM.  Scale bookkeeping:
    wq/wk/wv scaled x32 on host; u8 = u_ps/64 = 16u; scores = 16*s
    exp(sc/128) = exp(s/8);  a8 = 128*a/ssum (fp8-friendly range)
    vt8 = vt_ps/8 = 16*vtilde;  attn_ps = 512*attn -> x(1/512) on copy-out.
"""
import contextlib
import numpy as np
import ml_dtypes
import concourse.bacc as bacc
import concourse.tile as tile
import concourse.mybir as mybir
import concourse.bass as bass
from concourse import bass_utils

B, N, D, H = 64, 512, 1024, 16
DPH = D // H            # 64
NCORES = 8
BL = B // NCORES        # 8 batch rows per core
DDT = D // 128          # 8 d-tiles
EPS = 1e-5

f32 = mybir.dt.float32
f32r = mybir.dt.float32r
bf16 = mybir.dt.bfloat16
f8 = mybir.dt.float8e4
AF = mybir.ActivationFunctionType
AX = mybir.AxisListType
ALU = mybir.AluOpType
DR = mybir.MatmulPerfMode.DoubleRow


def _q8(x):
    return np.asarray(x, np.float32).astype(ml_dtypes.float8_e4m3)


def _emit(nc, tc, ap, ctx, repeat=1):
    const = ctx.enter_context(tc.tile_pool(name="const", bufs=1))
    wts = ctx.enter_context(tc.tile_pool(name="wts", bufs=1))
    once = ctx.enter_context(tc.tile_pool(name="once", bufs=1))
    io = ctx.enter_context(tc.tile_pool(name="io", bufs=BL))
    work = ctx.enter_context(tc.tile_pool(name="work", bufs=2))
    fpool = ctx.enter_context(tc.tile_pool(name="fpool", bufs=1))
    pbig = ctx.enter_context(tc.tile_pool(name="pbig", bufs=1, space="PSUM"))
    psc = ctx.enter_context(tc.tile_pool(name="psc", bufs=2, space="PSUM"))
    pmisc = ctx.enter_context(tc.tile_pool(name="pmisc", bufs=1, space="PSUM"))
    pvt = ctx.enter_context(tc.tile_pool(name="pvt", bufs=2, space="PSUM"))

    # ---- constants ----
    cstF = const.tile([128, 192], f32, tag="cstF")  # bqe | 128*I16 | id64
    qT8 = const.tile([128, 64], f8, tag="qT8")
    qres = const.tile([BL, D], f32, tag="qres")
    gb = const.tile([BL, 2 * D], bf16, tag="gb")
    nc.sync.dma_start(cstF[:], ap["cstF"][:])
    nc.sync.dma_start(qT8[:], ap["qT8"][:])
    bqe = cstF[:, 0:64]
    id128_16 = cstF[0:16, 64:80]
    id16 = cstF[0:16, 80:96]
    id64 = cstF[0:64, 128:192]

    # sqrt-table warm input (see dummy sqrt below)
    warm = once.tile([1, 1], f32, tag="warm")
    nc.vector.memset(warm[:], 1.0)

    wq8 = wts.tile([128, DDT, D], f8, tag="wq8")          # [p, dd, he]
    wk8 = wts.tile([128, 8, 1024], f8, tag="wk8")         # [(h%2)*64+j, h//2, dd*128+i]
    wv8 = wts.tile([128, H, DDT, DPH], f8, tag="wv8")     # [p, h, dd, e]
    u8 = once.tile([128, DDT, BL, H], f8, tag="u8")       # [p, dd, b, h] = 16*u
    vt8 = once.tile([128, DDT, H, BL], f8, tag="vt8")     # [p, dd, h, b] = 16*vtilde

    # ---- all input DMAs issued upfront (loads have no deps; DMA device
    # serializes in issue order). kT_b rides one v-slot ahead of v_b so each
    # b's score/softmax path overlaps v_b's transfer; v split in halves so
    # the first n-pair of vtilde matmuls starts mid-transfer. ----
    kv = []
    for b in range(BL):
        kt = io.tile([128, DDT, N], f8, tag="kt")
        vt = io.tile([128, 4, D], f8, tag="vt")
        kv.append((kt, vt))

    def dma_k(b):
        nc.sync.dma_start(kv[b][0][:], ap["kT8"][:, b * 4096:(b + 1) * 4096])

    def dma_v(b, half):
        nc.sync.dma_start(kv[b][1][:, 2 * half:2 * half + 2, :],
                          ap["v8"][:, b * 4096 + half * 2048:
                                    b * 4096 + (half + 1) * 2048])

    dma_k(0)
    nc.sync.dma_start(wq8[:], ap["wq8"][:])
    nc.sync.dma_start(wk8[:], ap["wk8"][:])
    dma_k(1)
    dma_v(0, 0)
    dma_v(0, 1)
    dma_k(2)
    dma_v(1, 0)
    dma_v(1, 1)
    nc.sync.dma_start(wv8[:], ap["wv8"][:])
    nc.sync.dma_start(qres[:], ap["qres"][:])
    nc.sync.dma_start(gb[:], ap["gb"][:])
    for b in range(3, BL):
        dma_k(b)
        dma_v(b - 1, 0)
        dma_v(b - 1, 1)
    dma_v(BL - 1, 0)
    dma_v(BL - 1, 1)

    # ---- bigT [128, dd, b, h] f32: serves three disjoint lifetimes ----
    # (1) q-proj scratch (cols of dd=0 block), (2) u_ps, (3) fin [8, 1024]
    bigT = pbig.tile([128, DDT, BL, H], f32, tag="big")

    # u8 = 16 * Wk_h^T (Wq_h q_b + bq_h); qT_ps view packs [p, hc*8+b] = 32q
    for hc in range(8):
        qv = bigT[:, 0, hc // 2, (hc % 2) * 8:(hc % 2) * 8 + 8]
        for dd in range(DDT):
            nc.tensor.matmul(qv,
                             wq8[:, dd, hc * 128:(hc + 1) * 128],
                             qT8[:, dd * 8:(dd + 1) * 8],
                             start=(dd == 0), stop=(dd == DDT - 1))
    q8sb = once.tile([128, 64], f8, tag="q8sb")
    nc.vector.tensor_add(q8sb[:], bigT[:, 0, 0:4, :], bqe)
    for h in range(H):
        po = (h % 2) * 64
        hp = h // 2
        for dd in range(DDT):
            nc.tensor.matmul(bigT[:, dd, :, h],
                             wk8[po:po + 64, hp, dd * 128:(dd + 1) * 128],
                             q8sb[po:po + 64, hp * 8:(hp + 1) * 8],
                             start=True, stop=True)
    nc.scalar.activation(u8[:], bigT[:], AF.Copy, scale=1.0 / 64)

    NITER = repeat * BL
    for it in range(NITER):
        b = it % BL
        kt, vt = kv[b]

        # scores [16, 512] = 16 * u_b . k_bn  (fp8 DoubleRow over d)
        sc = psc.tile([16, N], f32, tag="sc")
        for j in range(4):
            nc.tensor.matmul(sc[:], u8[:, 2 * j:2 * j + 2, b, :],
                             kt[:, 2 * j:2 * j + 2, :],
                             start=(j == 0), stop=(j == 3), perf_mode=DR)

        # softmax over n (no max-shift: |sc/128| <= ~0.5)
        a_sb = work.tile([16, N], f32, tag="a_sb")
        ssum = work.tile([16, 1], f32, tag="ssum")
        nc.scalar.activation(a_sb[:], sc[:], AF.Exp, scale=1.0 / 128,
                             accum_out=ssum[:])
        if it == NITER - 1:
            # dummy sqrt reading ssum: the data dep pins it after the last
            # exp, swapping the Act table to the sqrt set while Act is idle,
            # so the epilogue's Square/Sqrt pay no in-line table load
            nc.scalar.activation(warm[:], ssum[0:1, :], AF.Sqrt)
        rsum = work.tile([16, 1], f32, tag="rsum")
        nc.vector.reciprocal(rsum[:], ssum[:])
        nc.vector.tensor_scalar(a_sb[:], a_sb[:], rsum[:], 128.0,
                                ALU.mult, ALU.mult)

        # aT [n, t, h] via PE transpose
        pat = pmisc.tile([128, 64], f32, tag="pat")
        for t in range(4):
            nc.tensor.transpose(pat[:, t * 16:(t + 1) * 16],
                                a_sb[:, t * 128:(t + 1) * 128], id16)
        aT8 = work.tile([128, 4, 16], f8, tag="aT8")
        nc.vector.tensor_copy(aT8[:], pat[:])

        # vtilde^T [d, h] = sum_n values[n, d] a[n, h]  (values stationary)
        vt_ps = pvt.tile([128, DDT, H], f32, tag="vt_ps")   # 128*vtilde
        for dd in range(DDT):
            for tp in range(2):
                nc.tensor.matmul(vt_ps[:, dd, :],
                                 vt[:, 2 * tp:2 * tp + 2,
                                    dd * 128:(dd + 1) * 128],
                                 aT8[:, 2 * tp:2 * tp + 2, :],
                                 start=(tp == 0), stop=(tp == 1),
                                 perf_mode=DR)
        nc.scalar.activation(vt8[:, :, :, b], vt_ps[:], AF.Copy, scale=1.0 / 8)

    # stage 2: attn[64e, h*8+b] = sum_dd Wv_h[.,dd] vt8[.,dd,h,b], then
    # per-head f32 transposes gather into flat fin [8, 1024]
    attn_ps = pmisc.tile([64, 128], f32, tag="attn_ps")     # 512*attn
    for h in range(H):
        for dd in range(DDT):
            nc.tensor.matmul(attn_ps[:, h * 8:(h + 1) * 8],
                             wv8[:, h, dd, :], vt8[:, dd, h, :],
                             start=(dd == 0), stop=(dd == DDT - 1))
    attnTh = fpool.tile([64, 128], f32, tag="attnTh")
    nc.scalar.activation(attnTh[:], attn_ps[:], AF.Copy, scale=1.0 / 512)
    # fin [8, 1024] lives in bigT[0:8] (u_ps long dead); col h*64+e maps to
    # 4D (dd=h//2, b2=4*(h%2)+e//16, h2=e%16)
    for h in range(H):
        nc.tensor.transpose(bigT[0:8, h // 2, 4 * (h % 2):4 * (h % 2) + 4, :],
                            attnTh[:, h * 8:(h + 1) * 8], id64)
    fin = bigT[0:8, :, :, :]

    # ---- epilogue: residual + LayerNorm (bf16 elementwise, fp32 stats),
    # first [8,1024] passes split in halves to overlap DVE/Act/gather ----
    xbf = fpool.tile([BL, D], bf16, tag="xbf")
    nc.vector.tensor_add(xbf[:], fin, qres[:])
    msum = fpool.tile([BL, 1], f32, tag="msum")
    nc.vector.reduce_sum(msum[:], xbf[:], axis=AX.X)
    sq = fpool.tile([BL, D], bf16, tag="sq")
    vsum = fpool.tile([BL, 1], f32, tag="vsum")
    nc.scalar.activation(sq[:], xbf[:], AF.Square, accum_out=vsum[:])
    mu = fpool.tile([BL, 1], f32, tag="mu")
    nc.vector.tensor_scalar_mul(mu[:], msum[:], 1.0 / D)
    nmu2 = fpool.tile([BL, 1], f32, tag="nmu2")
    nc.vector.tensor_mul(nmu2[:], mu[:], mu[:])
    nc.vector.tensor_scalar(nmu2[:], nmu2[:], -1.0, EPS, ALU.mult, ALU.add)
    std = fpool.tile([BL, 1], f32, tag="std")
    nc.scalar.activation(std[:], vsum[:], AF.Sqrt, scale=1.0 / D, bias=nmu2[:])
    rstd = fpool.tile([BL, 1], f32, tag="rstd")
    nc.vector.reciprocal(rstd[:], std[:])
    negms = fpool.tile([BL, 1], f32, tag="negms")
    nc.vector.tensor_scalar(negms[:], rstd[:], mu[:], -1.0, ALU.mult, ALU.mult)
    ybf = fpool.tile([BL, D], bf16, tag="ybf")
    nc.vector.tensor_scalar(ybf[:], xbf[:], rstd[:], negms[:], ALU.mult, ALU.add)
    nc.vector.tensor_mul(ybf[:], ybf[:], gb[:, 0:D])
    xout = fpool.tile([BL, D], f32, tag="xout")
    nc.vector.tensor_add(xout[:], ybf[:], gb[:, D:2 * D])
    nc.sync.dma_start(ap["out"][:], xout[:])


_CACHED = {}


def _build(repeat=1):
    global _CACHED
    if repeat in _CACHED:
        return _CACHED[repeat]
    nc = bacc.Bacc("TRN2", target_bir_lowering=False, debug=False,
                   num_devices=NCORES)
    names = {}

    def di(name, shape, dt):
        names[name] = nc.dram_tensor(name, shape, dt, kind="ExternalInput").ap()

    di("kT8", [128, BL * DDT * N], f8)
    di("v8", [128, BL * 4 * D], f8)
    di("wq8", [128, DDT * D], f8)
    di("wk8", [128, 8 * 1024], f8)
    di("wv8", [128, H * DDT * DPH], f8)
    di("cstF", [128, 192], f32)
    di("qT8", [128, 64], f8)
    di("qres", [BL, D], f32)
    di("gb", [BL, 2 * D], bf16)
    names["out"] = nc.dram_tensor("out", [BL, D], f32, kind="ExternalOutput").ap()
    with tile.TileContext(nc) as tc:
        with contextlib.ExitStack() as ctx:
            _emit(nc, tc, names, ctx, repeat=repeat)
    nc.compile()
    _CACHED[repeat] = nc
    return nc


def _host_prep(queries, keys, values, Wq, bq, Wk, bk, Wv, bv, gamma, beta):
    queries = np.asarray(queries, np.float32)
    keys = np.asarray(keys, np.float32)
    values = np.asarray(values, np.float32)
    wq_f = np.asarray(Wq, np.float32).reshape(D, D)   # [he, d]
    wk_f = np.asarray(Wk, np.float32).reshape(D, D)
    wv_f = np.asarray(Wv, np.float32).reshape(D, D)
    bq_f = np.asarray(bq, np.float32).reshape(D)
    bv_f = np.asarray(bv, np.float32).reshape(D)
    gamma = np.asarray(gamma, np.float32).reshape(D)
    beta = np.asarray(beta, np.float32).reshape(D)

    wq8 = _q8((32 * wq_f).T.reshape(DDT, 128, D).transpose(1, 0, 2).reshape(128, -1))
    wk8 = _q8((32 * wk_f).reshape(8, 2, 64, D).transpose(1, 2, 0, 3).reshape(128, -1))
    wv8 = _q8((32 * wv_f).reshape(H, DPH, DDT, 128).transpose(3, 0, 2, 1)
              .reshape(128, -1))
    bqT = (32 * bq_f).reshape(DDT, 128).T                 # [128, hc]
    cstF = np.zeros((128, 192), np.float32)
    cstF[:, 0:64] = np.repeat(bqT[:, :, None], BL, axis=2).reshape(128, 64)
    cstF[0:16, 64:80] = 128.0 * np.eye(16, dtype=np.float32)
    cstF[0:16, 80:96] = np.eye(16, dtype=np.float32)
    cstF[0:64, 128:192] = np.eye(64, dtype=np.float32)
    gb_bf = np.concatenate([np.tile(gamma[None, :], (BL, 1)),
                            np.tile(beta[None, :], (BL, 1))], axis=1)
    gb_bf = gb_bf.astype(ml_dtypes.bfloat16)

    in_maps = []
    for c in range(NCORES):
        sl = slice(c * BL, (c + 1) * BL)
        # kT8: [p, b*4096 + dd*512 + n] = keys[b, n, dd*128+p]
        kT8 = _q8(keys[sl].reshape(BL, N, DDT, 128).transpose(3, 0, 2, 1)
                  .reshape(128, -1))
        # v8: [p, b*4096 + t*1024 + d] = values[b, t*128+p, d]
        v8 = _q8(values[sl].reshape(BL, 4, 128, D).transpose(2, 0, 1, 3)
                 .reshape(128, -1))
        # qT8: [p, dd*8 + b] = queries[b, dd*128+p]
        qT8 = _q8(queries[sl].reshape(BL, DDT, 128).transpose(2, 1, 0)
                  .reshape(128, -1))
        in_maps.append({
            "kT8": kT8, "v8": v8, "wq8": wq8, "wk8": wk8, "wv8": wv8,
            "cstF": cstF, "qT8": qT8,
            "qres": np.ascontiguousarray(queries[sl] + bv_f[None, :]),
            "gb": gb_bf,
        })
    return in_maps


def kernel(queries, keys, values, Wq, bq, Wk, bk, Wv, bv, gamma, beta):
    nc = _build()
    in_maps = _host_prep(queries, keys, values, Wq, bq, Wk, bk, Wv, bv,
                         gamma, beta)
    last_err = None
    for attempt in range(3):
        try:
            res = bass_utils.run_bass_kernel_spmd(nc, in_maps,
                                                  core_ids=list(range(NCORES)))
            return np.concatenate([r["out"] for r in res.results], axis=0)
        except Exception as e:  # transient NRT device errors: retry
            last_err = e
            import time as _time
            _time.sleep(5)
    raise last_err
